# revision 30
# baseline (speedup 1.0000x reference)
"""Trainium2 Bass kernel for nn_Attention_56470230008033.

Multi-head self-attention (B=2, N=2048, C=1024, H=16 heads, D=64),
k = v = q, full qkv projection + output projection.

Sharding over 8 NeuronCores: data parallel on batch (2) x tensor
parallel on heads (4 head-groups of 4 heads).

v5: ScalarE-exp-bound pipeline (~147us of ACTIVATE is the floor).
  - host pre-transposes x and pre-casts x/weights to bf16: no PE
    transposes, no DVE weight/x casts, input DMA halved (6MB)
  - pair-sequential units (8 pair-quarters x 16 key tiles): logits
    as one row-paired wall (2 heads via tile_position), ONE exp per
    unit (FD=1024, back-to-back 1.15us stream), PV as 2x [65,512]
    ones-column matmuls (denominator rides the 65th row; exactly one
    accumulation group per psum bank - multiple groups per bank are
    illegal, the hw start-flag clear is bank-granular)
  - PSUM: bp 2x[128,2,512] (exp double buffer) + cp 2x[65,512] +
    aux 2 = 8 banks
  - the tile framework does not track partition-sliced WRITES (the
    oT2 normalize muls): their readers get explicit add_dep_helper
    edges, as does the cp-bank reuse WAR across pair-quarters
  - input DMA split across sync/scalar/gpsimd rings, 2D descriptors,
    deadline-ordered; ScalarE's ring is idle before the exp stream
  - ~5us of dummy matmuls at boot warm the PE HAM clock gate
  - qkv/proj matmuls, v-units and output tiles run as deadline-paced
    fillers in the PE slack under the exp stream
"""

import sys

for _p in ("/opt/trn_rl_repo", "/opt/pypackages"):
    if _p not in sys.path:
        sys.path.append(_p)

import numpy as np
import ml_dtypes

B, N, C, H = 2, 2048, 1024, 16
D = C // H            # 64 head dim
NCORES = 8
HPC = 4               # heads per core
F = HPC * D           # 256 features per core
NT = N // 128         # 16 token tiles
CT = C // 128         # 8 contraction tiles

PVLAG = 6             # PV lag in key-tile units
NWARM = 48            # dummy matmuls to warm the PE clock gate

_CACHE = {}


def _build():
    from concourse import bacc, bass, mybir, tile
    from concourse.tile import add_dep_helper

    F32 = mybir.dt.float32
    BF16 = mybir.dt.bfloat16
    AF = mybir.ActivationFunctionType

    nc = bacc.Bacc(
        "TRN2",
        target_bir_lowering=False,
        debug=False,
        enable_asserts=False,
        num_devices=NCORES,
    )
    xt_d = nc.dram_tensor("xt", [C, N], BF16, kind="ExternalInput")
    wqk_d = nc.dram_tensor("wqk", [C, 2 * F], BF16, kind="ExternalInput")
    wv_d = nc.dram_tensor("wv", [C, F], BF16, kind="ExternalInput")
    wp_d = nc.dram_tensor("wp", [F, C], BF16, kind="ExternalInput")
    bqk_d = nc.dram_tensor("bqk", [2 * F, 1], F32, kind="ExternalInput")
    bv_d = nc.dram_tensor("bv", [1, F], F32, kind="ExternalInput")
    y_d = nc.dram_tensor("y", [N, C], BF16, kind="ExternalOutput")

    scale = float(D) ** -0.5

    with tile.TileContext(nc) as tc:
        from contextlib import ExitStack

        with ExitStack() as ctx:
            const = ctx.enter_context(tc.tile_pool(name="const", bufs=1))
            persist = ctx.enter_context(tc.tile_pool(name="persist", bufs=1))

            warm = const.tile([128, 512], BF16, name="warm", tag="warm")
            ones1 = const.tile([128, 1], BF16, name="ones1", tag="ones1")
            scr_in = const.tile([1, 16], F32, name="scr_in", tag="scr_in")
            scr = const.tile([1, 16], F32, name="scr", tag="scr")
            bqk_sb = const.tile([128, 4, 1], F32, name="bqk_sb", tag="bqk_sb")
            bv1 = const.tile([1, F], F32, name="bv1", tag="bv1")
            bvb = const.tile([128, F], F32, name="bvb", tag="bvb")

            # x^T, c-major: [p, c, tok] (qk rhs / v lhsT), bf16 direct DMA
            xT4 = persist.tile([128, CT, N], BF16, name="xT4", tag="xT4")
            # qkT[0..1] = Q^T head-pairs, qkT[2..3] = K^T head-pairs
            qkT = [persist.tile([128, N], BF16, name=f"qkT{f}", tag=f"qkT{f}")
                   for f in range(4)]
            vaug = [persist.tile([128, 65 * HPC], BF16, name=f"vaug{t}",
                                 tag=f"vaug{t}")
                    for t in range(NT)]
            # O^T stacked per head pair (rows 0-63 head 2p, 64-127 head 2p+1)
            oT2 = [persist.tile([128, N], BF16, name=f"oT2{p}", tag=f"oT2{p}")
                   for p in range(2)]
            wqk = persist.tile([128, CT, 2 * F], BF16, name="wqk", tag="wqk")
            wv = persist.tile([128, CT, F], BF16, name="wv", tag="wv")
            wp2 = persist.tile([128, 2, C], BF16, name="wp2", tag="wp2")

            ptp = ctx.enter_context(tc.tile_pool(name="ptp", bufs=16))
            ysb = ctx.enter_context(tc.tile_pool(name="ysb", bufs=2))
            snr = ctx.enter_context(tc.tile_pool(name="snr", bufs=6))
            snb = ctx.enter_context(tc.tile_pool(name="snb", bufs=2))
            sev = ctx.enter_context(tc.tile_pool(name="sev", bufs=8))

            bpp = ctx.enter_context(
                tc.tile_pool(name="bpp", bufs=2, space=bass.MemorySpace.PSUM))
            cpp = ctx.enter_context(
                tc.tile_pool(name="cpp", bufs=2, space=bass.MemorySpace.PSUM))
            aux = ctx.enter_context(
                tc.tile_pool(name="aux", bufs=2, space=bass.MemorySpace.PSUM))

            # ---------------- boot: warmup + exp table preload ----------
            nc.vector.memset(warm[:], 0.0)
            nc.vector.memset(ones1[:], 1.0)
            nc.vector.memset(scr_in[:], 0.0)
            nc.scalar.activation(scr[:], scr_in[:], AF.Exp)
            for _ in range(NWARM):
                wa = aux.tile([128, 512], F32, name="wa", tag="aux")
                nc.tensor.matmul(wa[:], warm[:, 0:128], warm[:],
                                 start=True, stop=True)

            # ---------------- DMA issue (deadline order per ring) -------
            xt = xt_d.ap().rearrange("(c p) n -> p c n", p=128)
            wqk_v = wqk_d.ap().rearrange("(c p) f -> p c f", p=128)
            wv_v = wv_d.ap().rearrange("(c p) f -> p c f", p=128)
            wp_v = wp_d.ap().rearrange("(t p) f -> p t f", p=128)
            bqk_v = bqk_d.ap().rearrange("(g p) o -> p g o", p=128)

            # NOTE: all transfers use 2D [128, n] dest slices — big 3D
            # strided DMAs were observed to break the DMA->reader RAW
            # dependency on hw (readers ran on stale SBUF)
            # scalar ring (HWDGE): tiny biases + prefix-critical wqk + ch1
            for g in range(4):
                nc.scalar.dma_start(bqk_sb[:, g], bqk_v[:, g])
            nc.scalar.dma_start(bv1[:], bv_d.ap()[:])
            for c in range(6):
                nc.scalar.dma_start(wqk[:, c], wqk_v[:, c])
            for c in range(4):
                nc.scalar.dma_start(xT4[:, c, 512:1024], xt[:, c, 512:1024])
            # sync ring (HWDGE): x first token chunk (c 0-3) + later chunks
            for c in range(4):
                nc.sync.dma_start(xT4[:, c, 0:512], xt[:, c, 0:512])
            for c in range(4):
                nc.sync.dma_start(xT4[:, c, 1024:1536], xt[:, c, 1024:1536])
            for c in range(4):
                nc.sync.dma_start(xT4[:, c, 1536:2048], xt[:, c, 1536:2048])
            # gpsimd ring (SWDGE): x (c 4-7), rest of wqk, wv, wp
            for c in range(4, 8):
                nc.gpsimd.dma_start(xT4[:, c, 0:512], xt[:, c, 0:512])
            for c in (6, 7):
                nc.gpsimd.dma_start(wqk[:, c], wqk_v[:, c])
            for c in range(4, 8):
                nc.gpsimd.dma_start(xT4[:, c, 512:1024], xt[:, c, 512:1024])
            for c in range(CT):
                nc.gpsimd.dma_start(wv[:, c], wv_v[:, c])
            for c in range(4, 8):
                nc.gpsimd.dma_start(xT4[:, c, 1024:1536], xt[:, c, 1024:1536])
            for c in range(4, 8):
                nc.gpsimd.dma_start(xT4[:, c, 1536:2048], xt[:, c, 1536:2048])
            for p in range(2):
                nc.gpsimd.dma_start(wp2[:, p], wp_v[:, p])
            nc.gpsimd.partition_broadcast(bvb[:], bv1[:])

            # ---------------- helper emitters ----------------
            def qk_unit(f, ch):
                # qkT[f][:, ch*512:(ch+1)*512] = (wqk_f^T @ x^T) + bias
                qp = aux.tile([128, 512], F32, name="qp", tag="aux")
                for c in (0, 4, 1, 5, 2, 6, 3, 7):
                    nc.tensor.matmul(
                        qp[:],
                        wqk[:, c, f * 128:(f + 1) * 128],
                        xT4[:, c, ch * 512:(ch + 1) * 512],
                        start=(c == 0), stop=(c == 7))
                nc.vector.tensor_scalar_add(
                    qkT[f][:, ch * 512:(ch + 1) * 512], qp[:],
                    bqk_sb[:, f, 0:1])

            vaug_w = {}

            def v_unit(t):
                # vaug[t] cols [65h,65h+64] = V head h | ones column.
                # The strided add write is invisible to the dep tracker
                # (like all non-plain writes): PV readers take manual
                # edges via vaug_w.
                vp = aux.tile([128, F], F32, name="vp", tag="aux")
                for c in range(CT):
                    nc.tensor.matmul(
                        vp[:], xT4[:, c, t * 128:(t + 1) * 128], wv[:, c],
                        start=(c == 0), stop=(c == CT - 1))
                ws = [nc.vector.memset(vaug[t][:, 65 * h + 64:65 * h + 65], 1.0)
                      for h in range(HPC)]
                vv = vaug[t].rearrange("p (h d) -> p h d", h=HPC)
                ws.append(nc.vector.tensor_add(
                    vv[:, :, 0:D],
                    vp.rearrange("p (h d) -> p h d", h=HPC),
                    bvb.rearrange("p (h d) -> p h d", h=HPC)))
                vaug_w[t] = ws

            ydma = [0]

            def yp_unit(t):
                # one DVE writer -> one DMA per ys tile, alternating
                # sync/gpsimd rings. The oT2 columns this reads were
                # written by partition-sliced muls the tracker can't
                # see, so add manual RAW edges.
                for ch in range(2):
                    yp = aux.tile([128, 512], F32, name="yp", tag="aux")
                    for p in range(2):
                        m = nc.tensor.matmul(
                            yp[:],
                            oT2[p][:, t * 128:(t + 1) * 128],
                            wp2[:, p, ch * 512:(ch + 1) * 512],
                            start=(p == 0), stop=(p == 1))
                        for mu in o_muls[t // 4]:
                            add_dep_helper(m.ins, mu.ins, reason="oT2->yp")
                    ys = ysb.tile([128, 512], BF16, name="ys", tag="ys")
                    nc.vector.tensor_copy(ys[:], yp[:])
                    eng = nc.sync if ydma[0] % 2 == 0 else nc.gpsimd
                    ydma[0] += 1
                    eng.dma_start(
                        y_d.ap()[t * 128:(t + 1) * 128,
                                 ch * 512:(ch + 1) * 512], ys[:])

            # -------- filler queue (deadline ordered) -----
            # slot = global key-tile index 0..63; fillers popped one per
            # slot plus forced pops when a deadline is due
            fillers = []

            def defer(dl, fn, *a, nb=0):
                fillers.append((dl, nb, lambda: fn(*a)))

            # deadlines in global unit slots (8 pair-quarters x 16):
            # K-pair0 chunks feed pq0, K-pair1 feed pq1; Q chunks feed
            # their pair-quarter; v tiles feed PV at lag; yp after the
            # producing quarter's second normalize (not_before)
            defer(3, qk_unit, 2, 1)
            defer(7, qk_unit, 2, 2)
            defer(11, qk_unit, 2, 3)
            defer(12, qk_unit, 3, 0)
            defer(14, qk_unit, 1, 0)
            defer(19, qk_unit, 3, 1)
            defer(23, qk_unit, 3, 2)
            defer(27, qk_unit, 3, 3)
            defer(30, qk_unit, 0, 1)
            defer(45, qk_unit, 1, 1)
            defer(62, qk_unit, 0, 2)
            defer(77, qk_unit, 1, 2)
            defer(94, qk_unit, 0, 3)
            defer(109, qk_unit, 1, 3)
            for t in range(NT):
                defer(min(t + PVLAG - 1, 15), v_unit, t)
            for t in range(12):
                defer(32 * (t // 4 + 1) + 8 + (t % 4), yp_unit, t,
                      nb=32 * (t // 4 + 1))
            fillers.sort(key=lambda x: x[0])

            def emit_fillers(slot):
                popped = False
                while fillers:
                    ready = [f for f in fillers if f[1] <= slot]
                    if not ready:
                        break
                    due = [f for f in ready if f[0] <= slot]
                    if due:
                        f = due[0]
                    elif not popped:
                        f = ready[0]
                    else:
                        break
                    fillers.remove(f)
                    f[2]()
                    popped = True

            # ---------------- prefix: K/Q chunk 0, pair 0 ----------
            qk_unit(2, 0)
            qk_unit(0, 0)

            # ---------------- fused attention ----------------
            # Exactly ONE psum accumulation group per bank (multiple
            # start=True groups in one bank are illegal: the clear is
            # bank-granular). Denominators ride the ones column as row
            # 64 of each [65, 512] accumulator.
            # The tracker is blind to PARTITION-SLICED WRITES (the oT2
            # muls): their readers (yp matmuls) and the cp-bank reuse
            # WAR get explicit add_dep_helper edges.
            cp_mms = [[], []]
            o_muls = {}
            war_deps = []

            def pv_unit(ent, cph, cphp, pair):
                pt, mt = ent
                st, sp = (mt == 0), (mt == NT - 1)
                he = 2 * pair
                m0 = nc.tensor.matmul(
                    cph[:], vaug[mt][:, 65 * he:65 * he + 65], pt[:, 0],
                    start=st, stop=sp)
                m1 = nc.tensor.matmul(
                    cphp[:], vaug[mt][:, 65 * he + 65:65 * he + 130], pt[:, 1],
                    start=st, stop=sp)
                for m in (m0, m1):
                    for w in vaug_w[mt]:
                        add_dep_helper(m.ins, w.ins, reason="vaug->pv")
                    if mt == 0:
                        for w in war_deps:
                            add_dep_helper(m.ins, w.ins, reason="evac->next")
                cp_mms[0].append(m0)
                cp_mms[1].append(m1)

            def normalize(quarter, pair, qs, cph, cphp):
                # evacuate both accumulators to sbuf with full-tile
                # copies (tracked), then normalize from sbuf
                sc = [sev.tile([65, 512], F32, name=f"sc{i}", tag="sev")
                      for i in range(2)]
                i_sc = [nc.vector.tensor_copy(sc[0][:], cph[:]),
                        nc.vector.tensor_copy(sc[1][:], cphp[:])]
                for i in range(2):
                    for m in cp_mms[i]:
                        add_dep_helper(i_sc[i].ins, m.ins, reason="cp->evac")
                muls = o_muls.setdefault(quarter, [])
                for i in range(2):
                    h = 2 * pair + i
                    rb = 64 * (h % 2)
                    d0 = snr.tile([1, 512], F32, name="d0", tag="sr")
                    nc.vector.tensor_copy(d0[:], sc[i][64:65, :])
                    sr = snr.tile([1, 512], F32, name="sr", tag="sr")
                    nc.vector.reciprocal_approx_fast(sr[:], d0[:])
                    sb = snb.tile([128, 512], F32, name="sb", tag="sb")
                    nc.gpsimd.partition_broadcast(sb[:], sr[:])
                    muls.append(nc.vector.tensor_mul(
                        oT2[h // 2][rb:rb + 64, qs:qs + 512],
                        sc[i][0:64, :], sb[0:64, :]))
                war_deps[:] = [i_sc[0], i_sc[1]]
                cp_mms[0] = []
                cp_mms[1] = []

            for pq in range(8):
                quarter, pair = divmod(pq, 2)
                qs = quarter * 512
                cph = cpp.tile([65, 512], F32, name="cph", tag="cp")
                cphp = cpp.tile([65, 512], F32, name="cphp", tag="cp")
                pts = []
                for mt in range(NT):
                    slot = pq * NT + mt
                    bp = bpp.tile([128, 2, 512], F32, name="bp", tag="bp")
                    nc.tensor.matmul(
                        bp[:, 0], qkT[2 + pair][0:64, mt * 128:(mt + 1) * 128],
                        qkT[pair][0:64, qs:qs + 512], start=True, stop=True)
                    nc.tensor.matmul(
                        bp[:, 1], qkT[2 + pair][64:128, mt * 128:(mt + 1) * 128],
                        qkT[pair][64:128, qs:qs + 512], start=True, stop=True)
                    pt = ptp.tile([128, 2, 512], BF16, name="pt", tag="pt")
                    nc.scalar.activation(pt[:], bp[:], AF.Exp, scale=scale)
                    pts.append((pt, mt))
                    if len(pts) > PVLAG:
                        pv_unit(pts.pop(0), cph, cphp, pair)
                    emit_fillers(slot)
                while pts:
                    pv_unit(pts.pop(0), cph, cphp, pair)
                normalize(quarter, pair, qs, cph, cphp)

            # tail
            while fillers:
                fillers.pop(0)[2]()
            for t in range(12, 16):
                yp_unit(t)

    nc.compile()
    return nc


def _get_nc():
    if "nc" not in _CACHE:
        _CACHE["nc"] = _build()
    return _CACHE["nc"]


def _in_maps(q, W_qkv, b_qkv, W_proj):
    bf16 = ml_dtypes.bfloat16
    # shared across cores: x^T per batch, per-group weight slices
    xts = [np.ascontiguousarray(np.asarray(q[b]).T).astype(bf16)
           for b in range(B)]
    wqks, wvs, wps, bqks, bvs = [], [], [], [], []
    for g in range(HPC):
        cols = slice(g * F, (g + 1) * F)
        wqks.append(np.ascontiguousarray(
            np.concatenate([W_qkv[:, cols], W_qkv[:, C:2 * C][:, cols]],
                           axis=1)).astype(bf16))
        wvs.append(np.ascontiguousarray(W_qkv[:, 2 * C:][:, cols]).astype(bf16))
        wps.append(np.ascontiguousarray(W_proj[cols, :]).astype(bf16))
        bqks.append(np.ascontiguousarray(
            np.concatenate([b_qkv[cols], b_qkv[C:2 * C][cols]])
            .reshape(2 * F, 1).astype(np.float32)))
        bvs.append(np.ascontiguousarray(
            b_qkv[2 * C:][cols].reshape(1, F).astype(np.float32)))
    maps = []
    for core in range(NCORES):
        b, g = divmod(core, HPC)
        maps.append({
            "xt": xts[b],
            "wqk": wqks[g],
            "wv": wvs[g],
            "wp": wps[g],
            "bqk": bqks[g],
            "bv": bvs[g],
        })
    return maps


def kernel(q, W_qkv, b_qkv, W_proj, b_proj):
    from concourse.bass_utils import run_bass_kernel_spmd

    q = np.asarray(q, dtype=np.float32)
    W_qkv = np.asarray(W_qkv, dtype=np.float32)
    b_qkv = np.asarray(b_qkv, dtype=np.float32)
    W_proj = np.asarray(W_proj, dtype=np.float32)
    b_proj = np.asarray(b_proj, dtype=np.float32)

    nc = _get_nc()
    res = run_bass_kernel_spmd(nc, _in_maps(q, W_qkv, b_qkv, W_proj),
                               core_ids=list(range(NCORES)))

    out = np.zeros((B, N, C), dtype=np.float32)
    for core in range(NCORES):
        out[core // HPC] += np.asarray(res.results[core]["y"], dtype=np.float32)
    out += b_proj
    return out


# revision 31
# speedup vs baseline: 1.0214x; 1.0214x over previous
"""Trainium2 Bass kernel for nn_Attention_56470230008033.

Multi-head self-attention (B=2, N=2048, C=1024, H=16 heads, D=64),
k = v = q, full qkv projection + output projection.

Sharding over 8 NeuronCores: data parallel on batch (2) x tensor
parallel on heads (4 head-groups of 4 heads).

v5: ScalarE-exp-bound pipeline (~147us of ACTIVATE is the floor).
  - host pre-transposes x and pre-casts x/weights to bf16: no PE
    transposes, no DVE weight/x casts, input DMA halved (6MB)
  - pair-sequential units (8 pair-quarters x 16 key tiles): logits
    as one row-paired wall (2 heads via tile_position), ONE exp per
    unit (FD=1024, back-to-back 1.15us stream), PV as 2x [65,512]
    ones-column matmuls (denominator rides the 65th row; exactly one
    accumulation group per psum bank - multiple groups per bank are
    illegal, the hw start-flag clear is bank-granular)
  - PSUM: bp 2x[128,2,512] (exp double buffer) + cp 2x[65,512] +
    aux 2 = 8 banks
  - the tile framework does not track partition-sliced WRITES (the
    oT2 normalize muls): their readers get explicit add_dep_helper
    edges, as does the cp-bank reuse WAR across pair-quarters
  - input DMA split across sync/scalar/gpsimd rings, 2D descriptors,
    deadline-ordered; ScalarE's ring is idle before the exp stream
  - ~5us of dummy matmuls at boot warm the PE HAM clock gate
  - qkv/proj matmuls, v-units and output tiles run as deadline-paced
    fillers in the PE slack under the exp stream
"""

import sys

for _p in ("/opt/trn_rl_repo", "/opt/pypackages"):
    if _p not in sys.path:
        sys.path.append(_p)

import numpy as np
import ml_dtypes

B, N, C, H = 2, 2048, 1024, 16
D = C // H            # 64 head dim
NCORES = 8
HPC = 4               # heads per core
F = HPC * D           # 256 features per core
NT = N // 128         # 16 token tiles
CT = C // 128         # 8 contraction tiles

PVLAG = 6             # PV lag in key-tile units
NWARM = 48            # dummy matmuls to warm the PE clock gate

_CACHE = {}


def _build():
    from concourse import bacc, bass, mybir, tile
    from concourse.tile import add_dep_helper

    F32 = mybir.dt.float32
    BF16 = mybir.dt.bfloat16
    AF = mybir.ActivationFunctionType

    nc = bacc.Bacc(
        "TRN2",
        target_bir_lowering=False,
        debug=False,
        enable_asserts=False,
        num_devices=NCORES,
    )
    xt_d = nc.dram_tensor("xt", [C, N], BF16, kind="ExternalInput")
    wqk_d = nc.dram_tensor("wqk", [C, 2 * F], BF16, kind="ExternalInput")
    wv_d = nc.dram_tensor("wv", [C, F], BF16, kind="ExternalInput")
    wp_d = nc.dram_tensor("wp", [F, C], BF16, kind="ExternalInput")
    bqk_d = nc.dram_tensor("bqk", [2 * F, 1], F32, kind="ExternalInput")
    bv_d = nc.dram_tensor("bv", [1, F], F32, kind="ExternalInput")
    y_d = nc.dram_tensor("y", [N, C], BF16, kind="ExternalOutput")

    scale = float(D) ** -0.5

    with tile.TileContext(nc) as tc:
        from contextlib import ExitStack

        with ExitStack() as ctx:
            const = ctx.enter_context(tc.tile_pool(name="const", bufs=1))
            persist = ctx.enter_context(tc.tile_pool(name="persist", bufs=1))

            warm = const.tile([128, 512], BF16, name="warm", tag="warm")
            ones1 = const.tile([128, 1], BF16, name="ones1", tag="ones1")
            scr_in = const.tile([1, 16], F32, name="scr_in", tag="scr_in")
            scr = const.tile([1, 16], F32, name="scr", tag="scr")
            bqk_sb = const.tile([128, 4, 1], F32, name="bqk_sb", tag="bqk_sb")
            bv1 = const.tile([1, F], F32, name="bv1", tag="bv1")
            bvb = const.tile([128, F], F32, name="bvb", tag="bvb")

            # x^T, c-major: [p, c, tok] (qk rhs / v lhsT), bf16 direct DMA
            xT4 = persist.tile([128, CT, N], BF16, name="xT4", tag="xT4")
            # qkT[0..1] = Q^T head-pairs, qkT[2..3] = K^T head-pairs
            qkT = [persist.tile([128, N], BF16, name=f"qkT{f}", tag=f"qkT{f}")
                   for f in range(4)]
            vaug = [persist.tile([128, 65 * HPC], BF16, name=f"vaug{t}",
                                 tag=f"vaug{t}")
                    for t in range(NT)]
            # O^T stacked per head pair (rows 0-63 head 2p, 64-127 head 2p+1)
            oT2 = [persist.tile([128, N], BF16, name=f"oT2{p}", tag=f"oT2{p}")
                   for p in range(2)]
            wqk = persist.tile([128, CT, 2 * F], BF16, name="wqk", tag="wqk")
            wv = persist.tile([128, CT, F], BF16, name="wv", tag="wv")
            wp2 = persist.tile([128, 2, C], BF16, name="wp2", tag="wp2")

            ptp = ctx.enter_context(tc.tile_pool(name="ptp", bufs=16))
            ysb = ctx.enter_context(tc.tile_pool(name="ysb", bufs=2))
            snr = ctx.enter_context(tc.tile_pool(name="snr", bufs=6))
            snb = ctx.enter_context(tc.tile_pool(name="snb", bufs=2))
            sev = ctx.enter_context(tc.tile_pool(name="sev", bufs=8))

            bpp = ctx.enter_context(
                tc.tile_pool(name="bpp", bufs=2, space=bass.MemorySpace.PSUM))
            cpp = ctx.enter_context(
                tc.tile_pool(name="cpp", bufs=2, space=bass.MemorySpace.PSUM))
            aux = ctx.enter_context(
                tc.tile_pool(name="aux", bufs=2, space=bass.MemorySpace.PSUM))

            # ---------------- boot: warmup + exp table preload ----------
            nc.vector.memset(warm[:], 0.0)
            nc.vector.memset(ones1[:], 1.0)
            nc.vector.memset(scr_in[:], 0.0)
            nc.scalar.activation(scr[:], scr_in[:], AF.Exp)
            for _ in range(NWARM):
                wa = aux.tile([128, 512], F32, name="wa", tag="aux")
                nc.tensor.matmul(wa[:], warm[:, 0:128], warm[:],
                                 start=True, stop=True)

            # ---------------- DMA issue (deadline order per ring) -------
            xt = xt_d.ap().rearrange("(c p) n -> p c n", p=128)
            wqk_v = wqk_d.ap().rearrange("(c p) f -> p c f", p=128)
            wv_v = wv_d.ap().rearrange("(c p) f -> p c f", p=128)
            wp_v = wp_d.ap().rearrange("(t p) f -> p t f", p=128)
            bqk_v = bqk_d.ap().rearrange("(g p) o -> p g o", p=128)

            # NOTE: all transfers use 2D [128, n] dest slices — big 3D
            # strided DMAs were observed to break the DMA->reader RAW
            # dependency on hw (readers ran on stale SBUF)
            # scalar ring (HWDGE): tiny biases + prefix-critical wqk + ch1
            for g in range(4):
                nc.scalar.dma_start(bqk_sb[:, g], bqk_v[:, g])
            nc.scalar.dma_start(bv1[:], bv_d.ap()[:])
            for c in range(6):
                nc.scalar.dma_start(wqk[:, c], wqk_v[:, c])
            for c in range(4):
                nc.scalar.dma_start(xT4[:, c, 512:1024], xt[:, c, 512:1024])
            # sync ring (HWDGE): x first token chunk (c 0-3) + later chunks
            for c in range(4):
                nc.sync.dma_start(xT4[:, c, 0:512], xt[:, c, 0:512])
            for c in range(4):
                nc.sync.dma_start(xT4[:, c, 1024:1536], xt[:, c, 1024:1536])
            for c in range(4):
                nc.sync.dma_start(xT4[:, c, 1536:2048], xt[:, c, 1536:2048])
            # gpsimd ring (SWDGE): x (c 4-7), rest of wqk, wv, wp
            for c in range(4, 8):
                nc.gpsimd.dma_start(xT4[:, c, 0:512], xt[:, c, 0:512])
            for c in (6, 7):
                nc.gpsimd.dma_start(wqk[:, c], wqk_v[:, c])
            for c in range(4, 8):
                nc.gpsimd.dma_start(xT4[:, c, 512:1024], xt[:, c, 512:1024])
            for c in range(CT):
                nc.gpsimd.dma_start(wv[:, c], wv_v[:, c])
            for c in range(4, 8):
                nc.gpsimd.dma_start(xT4[:, c, 1024:1536], xt[:, c, 1024:1536])
            for c in range(4, 8):
                nc.gpsimd.dma_start(xT4[:, c, 1536:2048], xt[:, c, 1536:2048])
            for p in range(2):
                nc.gpsimd.dma_start(wp2[:, p], wp_v[:, p])
            nc.gpsimd.partition_broadcast(bvb[:], bv1[:])

            # ---------------- helper emitters ----------------
            def qk_unit(f, ch):
                # qkT[f][:, ch*512:(ch+1)*512] = (wqk_f^T @ x^T) + bias
                qp = aux.tile([128, 512], F32, name="qp", tag="aux")
                for c in (0, 4, 1, 5, 2, 6, 3, 7):
                    nc.tensor.matmul(
                        qp[:],
                        wqk[:, c, f * 128:(f + 1) * 128],
                        xT4[:, c, ch * 512:(ch + 1) * 512],
                        start=(c == 0), stop=(c == 7))
                nc.vector.tensor_scalar_add(
                    qkT[f][:, ch * 512:(ch + 1) * 512], qp[:],
                    bqk_sb[:, f, 0:1])

            vaug_w = {}

            def v_unit(t):
                # vaug[t] cols [65h,65h+64] = V head h | ones column.
                # The strided add write is invisible to the dep tracker
                # (like all non-plain writes): PV readers take manual
                # edges via vaug_w.
                vp = aux.tile([128, F], F32, name="vp", tag="aux")
                for c in range(CT):
                    nc.tensor.matmul(
                        vp[:], xT4[:, c, t * 128:(t + 1) * 128], wv[:, c],
                        start=(c == 0), stop=(c == CT - 1))
                ws = [nc.vector.memset(vaug[t][:, 65 * h + 64:65 * h + 65], 1.0)
                      for h in range(HPC)]
                vv = vaug[t].rearrange("p (h d) -> p h d", h=HPC)
                ws.append(nc.vector.tensor_add(
                    vv[:, :, 0:D],
                    vp.rearrange("p (h d) -> p h d", h=HPC),
                    bvb.rearrange("p (h d) -> p h d", h=HPC)))
                vaug_w[t] = ws

            def yp_unit(t):
                # one DVE writer -> one sync-ring DMA per ys tile (the
                # baseline-proven output pattern). The oT2 columns this
                # reads were written by partition-sliced muls the
                # tracker can't see, so add manual RAW edges.
                for ch in range(2):
                    yp = aux.tile([128, 512], F32, name="yp", tag="aux")
                    for p in range(2):
                        m = nc.tensor.matmul(
                            yp[:],
                            oT2[p][:, t * 128:(t + 1) * 128],
                            wp2[:, p, ch * 512:(ch + 1) * 512],
                            start=(p == 0), stop=(p == 1))
                        for mu in o_muls[t // 4]:
                            add_dep_helper(m.ins, mu.ins, reason="oT2->yp")
                    ys = ysb.tile([128, 512], BF16, name="ys", tag="ys")
                    nc.vector.tensor_copy(ys[:], yp[:])
                    nc.sync.dma_start(
                        y_d.ap()[t * 128:(t + 1) * 128,
                                 ch * 512:(ch + 1) * 512], ys[:])

            # -------- filler queue (deadline ordered) -----
            # slot = global key-tile index 0..63; fillers popped one per
            # slot plus forced pops when a deadline is due
            fillers = []

            def defer(dl, fn, *a, nb=0):
                fillers.append((dl, nb, lambda: fn(*a)))

            # deadlines in global unit slots (8 pair-quarters x 16):
            # K-pair0 chunks feed pq0, K-pair1 feed pq1; Q chunks feed
            # their pair-quarter; v tiles feed PV at lag; yp after the
            # producing quarter's second normalize (not_before)
            defer(3, qk_unit, 2, 1)
            defer(7, qk_unit, 2, 2)
            defer(11, qk_unit, 2, 3)
            defer(12, qk_unit, 3, 0)
            defer(14, qk_unit, 1, 0)
            defer(19, qk_unit, 3, 1)
            defer(23, qk_unit, 3, 2)
            defer(27, qk_unit, 3, 3)
            defer(30, qk_unit, 0, 1)
            defer(45, qk_unit, 1, 1)
            defer(62, qk_unit, 0, 2)
            defer(77, qk_unit, 1, 2)
            defer(94, qk_unit, 0, 3)
            defer(109, qk_unit, 1, 3)
            for t in range(NT):
                defer(min(t + PVLAG - 1, 15), v_unit, t)
            for t in range(12):
                defer(32 * (t // 4 + 1) + 8 + (t % 4), yp_unit, t,
                      nb=32 * (t // 4 + 1))
            fillers.sort(key=lambda x: x[0])

            def emit_fillers(slot):
                popped = False
                while fillers:
                    ready = [f for f in fillers if f[1] <= slot]
                    if not ready:
                        break
                    due = [f for f in ready if f[0] <= slot]
                    if due:
                        f = due[0]
                    elif not popped:
                        f = ready[0]
                    else:
                        break
                    fillers.remove(f)
                    f[2]()
                    popped = True

            # ---------------- prefix: K/Q chunk 0, pair 0 ----------
            qk_unit(2, 0)
            qk_unit(0, 0)

            # ---------------- fused attention ----------------
            # Exactly ONE psum accumulation group per bank (multiple
            # start=True groups in one bank are illegal: the clear is
            # bank-granular). Denominators ride the ones column as row
            # 64 of each [65, 512] accumulator.
            # The tracker is blind to PARTITION-SLICED WRITES (the oT2
            # muls): their readers (yp matmuls) and the cp-bank reuse
            # WAR get explicit add_dep_helper edges.
            cp_mms = [[], []]
            o_muls = {}
            war_deps = []

            def pv_unit(ent, cph, cphp, pair):
                pt, mt = ent
                st, sp = (mt == 0), (mt == NT - 1)
                he = 2 * pair
                m0 = nc.tensor.matmul(
                    cph[:], vaug[mt][:, 65 * he:65 * he + 65], pt[:, 0],
                    start=st, stop=sp)
                m1 = nc.tensor.matmul(
                    cphp[:], vaug[mt][:, 65 * he + 65:65 * he + 130], pt[:, 1],
                    start=st, stop=sp)
                for m in (m0, m1):
                    for w in vaug_w[mt]:
                        add_dep_helper(m.ins, w.ins, reason="vaug->pv")
                    if mt == 0:
                        for w in war_deps:
                            add_dep_helper(m.ins, w.ins, reason="evac->next")
                cp_mms[0].append(m0)
                cp_mms[1].append(m1)

            def normalize(quarter, pair, qs, cph, cphp):
                # evacuate both accumulators to sbuf with full-tile
                # copies (tracked), then normalize from sbuf
                sc = [sev.tile([65, 512], F32, name=f"sc{i}", tag="sev")
                      for i in range(2)]
                i_sc = [nc.vector.tensor_copy(sc[0][:], cph[:]),
                        nc.vector.tensor_copy(sc[1][:], cphp[:])]
                for i in range(2):
                    for m in cp_mms[i]:
                        add_dep_helper(i_sc[i].ins, m.ins, reason="cp->evac")
                muls = o_muls.setdefault(quarter, [])
                for i in range(2):
                    h = 2 * pair + i
                    rb = 64 * (h % 2)
                    d0 = snr.tile([1, 512], F32, name="d0", tag="sr")
                    nc.vector.tensor_copy(d0[:], sc[i][64:65, :])
                    sr = snr.tile([1, 512], F32, name="sr", tag="sr")
                    nc.vector.reciprocal_approx_fast(sr[:], d0[:])
                    sb = snb.tile([128, 512], F32, name="sb", tag="sb")
                    nc.gpsimd.partition_broadcast(sb[:], sr[:])
                    muls.append(nc.vector.tensor_mul(
                        oT2[h // 2][rb:rb + 64, qs:qs + 512],
                        sc[i][0:64, :], sb[0:64, :]))
                war_deps[:] = [i_sc[0], i_sc[1]]
                cp_mms[0] = []
                cp_mms[1] = []

            for pq in range(8):
                quarter, pair = divmod(pq, 2)
                qs = quarter * 512
                cph = cpp.tile([65, 512], F32, name="cph", tag="cp")
                cphp = cpp.tile([65, 512], F32, name="cphp", tag="cp")
                pts = []
                for mt in range(NT):
                    slot = pq * NT + mt
                    bp = bpp.tile([128, 2, 512], F32, name="bp", tag="bp")
                    nc.tensor.matmul(
                        bp[:, 0], qkT[2 + pair][0:64, mt * 128:(mt + 1) * 128],
                        qkT[pair][0:64, qs:qs + 512], start=True, stop=True)
                    nc.tensor.matmul(
                        bp[:, 1], qkT[2 + pair][64:128, mt * 128:(mt + 1) * 128],
                        qkT[pair][64:128, qs:qs + 512], start=True, stop=True)
                    pt = ptp.tile([128, 2, 512], BF16, name="pt", tag="pt")
                    nc.scalar.activation(pt[:], bp[:], AF.Exp, scale=scale)
                    pts.append((pt, mt))
                    if len(pts) > PVLAG:
                        pv_unit(pts.pop(0), cph, cphp, pair)
                    emit_fillers(slot)
                while pts:
                    pv_unit(pts.pop(0), cph, cphp, pair)
                normalize(quarter, pair, qs, cph, cphp)

            # tail
            while fillers:
                fillers.pop(0)[2]()
            for t in range(12, 16):
                yp_unit(t)

    nc.compile()
    return nc


def _get_nc():
    if "nc" not in _CACHE:
        _CACHE["nc"] = _build()
    return _CACHE["nc"]


def _in_maps(q, W_qkv, b_qkv, W_proj):
    bf16 = ml_dtypes.bfloat16
    # shared across cores: x^T per batch, per-group weight slices
    xts = [np.ascontiguousarray(np.asarray(q[b]).T).astype(bf16)
           for b in range(B)]
    wqks, wvs, wps, bqks, bvs = [], [], [], [], []
    for g in range(HPC):
        cols = slice(g * F, (g + 1) * F)
        wqks.append(np.ascontiguousarray(
            np.concatenate([W_qkv[:, cols], W_qkv[:, C:2 * C][:, cols]],
                           axis=1)).astype(bf16))
        wvs.append(np.ascontiguousarray(W_qkv[:, 2 * C:][:, cols]).astype(bf16))
        wps.append(np.ascontiguousarray(W_proj[cols, :]).astype(bf16))
        bqks.append(np.ascontiguousarray(
            np.concatenate([b_qkv[cols], b_qkv[C:2 * C][cols]])
            .reshape(2 * F, 1).astype(np.float32)))
        bvs.append(np.ascontiguousarray(
            b_qkv[2 * C:][cols].reshape(1, F).astype(np.float32)))
    maps = []
    for core in range(NCORES):
        b, g = divmod(core, HPC)
        maps.append({
            "xt": xts[b],
            "wqk": wqks[g],
            "wv": wvs[g],
            "wp": wps[g],
            "bqk": bqks[g],
            "bv": bvs[g],
        })
    return maps


def kernel(q, W_qkv, b_qkv, W_proj, b_proj):
    from concourse.bass_utils import run_bass_kernel_spmd

    q = np.asarray(q, dtype=np.float32)
    W_qkv = np.asarray(W_qkv, dtype=np.float32)
    b_qkv = np.asarray(b_qkv, dtype=np.float32)
    W_proj = np.asarray(W_proj, dtype=np.float32)
    b_proj = np.asarray(b_proj, dtype=np.float32)

    nc = _get_nc()
    res = run_bass_kernel_spmd(nc, _in_maps(q, W_qkv, b_qkv, W_proj),
                               core_ids=list(range(NCORES)))

    out = np.zeros((B, N, C), dtype=np.float32)
    for core in range(NCORES):
        out[core // HPC] += np.asarray(res.results[core]["y"], dtype=np.float32)
    out += b_proj
    return out


# revision 32
# speedup vs baseline: 1.0462x; 1.0242x over previous
"""Trainium2 Bass kernel for nn_Attention_56470230008033.

Multi-head self-attention (B=2, N=2048, C=1024, H=16 heads, D=64),
k = v = q, full qkv projection + output projection.

Sharding over 8 NeuronCores: data parallel on batch (2) x tensor
parallel on heads (4 head-groups of 4 heads).

v5: ScalarE-exp-bound pipeline (~147us of ACTIVATE is the floor).
  - host pre-transposes x and pre-casts x/weights to bf16: no PE
    transposes, no DVE weight/x casts, input DMA halved (6MB)
  - pair-sequential units (8 pair-quarters x 16 key tiles): logits
    as one row-paired wall (2 heads via tile_position), ONE exp per
    unit (FD=1024, back-to-back 1.15us stream), PV as 2x [65,512]
    ones-column matmuls (denominator rides the 65th row; exactly one
    accumulation group per psum bank - multiple groups per bank are
    illegal, the hw start-flag clear is bank-granular)
  - PSUM: bp 2x[128,2,512] (exp double buffer) + cp 2x[65,512] +
    aux 2 = 8 banks
  - the tile framework does not track partition-sliced WRITES (the
    oT2 normalize muls): their readers get explicit add_dep_helper
    edges, as does the cp-bank reuse WAR across pair-quarters
  - input DMA split across sync/scalar/gpsimd rings, 2D descriptors,
    deadline-ordered; ScalarE's ring is idle before the exp stream
  - ~5us of dummy matmuls at boot warm the PE HAM clock gate
  - qkv/proj matmuls, v-units and output tiles run as deadline-paced
    fillers in the PE slack under the exp stream
"""

import sys

for _p in ("/opt/trn_rl_repo", "/opt/pypackages"):
    if _p not in sys.path:
        sys.path.append(_p)

import numpy as np
import ml_dtypes

B, N, C, H = 2, 2048, 1024, 16
D = C // H            # 64 head dim
NCORES = 8
HPC = 4               # heads per core
F = HPC * D           # 256 features per core
NT = N // 128         # 16 token tiles
CT = C // 128         # 8 contraction tiles

PVLAG = 6             # PV lag in key-tile units
NWARM = 48            # dummy matmuls to warm the PE clock gate

_CACHE = {}


def _build():
    from concourse import bacc, bass, mybir, tile
    from concourse.tile import add_dep_helper

    F32 = mybir.dt.float32
    BF16 = mybir.dt.bfloat16
    AF = mybir.ActivationFunctionType

    nc = bacc.Bacc(
        "TRN2",
        target_bir_lowering=False,
        debug=False,
        enable_asserts=False,
        num_devices=NCORES,
    )
    xt_d = nc.dram_tensor("xt", [C, N], BF16, kind="ExternalInput")
    wqk_d = nc.dram_tensor("wqk", [C, 2 * F], BF16, kind="ExternalInput")
    wv_d = nc.dram_tensor("wv", [C, F], BF16, kind="ExternalInput")
    wp_d = nc.dram_tensor("wp", [F, C], BF16, kind="ExternalInput")
    bqk_d = nc.dram_tensor("bqk", [2 * F, 1], F32, kind="ExternalInput")
    bv_d = nc.dram_tensor("bv", [1, F], F32, kind="ExternalInput")
    y_d = nc.dram_tensor("y", [N, C], BF16, kind="ExternalOutput")

    scale = float(D) ** -0.5

    with tile.TileContext(nc) as tc:
        from contextlib import ExitStack

        with ExitStack() as ctx:
            const = ctx.enter_context(tc.tile_pool(name="const", bufs=1))
            persist = ctx.enter_context(tc.tile_pool(name="persist", bufs=1))

            warm = const.tile([128, 512], BF16, name="warm", tag="warm")
            ones1 = const.tile([128, 1], BF16, name="ones1", tag="ones1")
            scr_in = const.tile([1, 16], F32, name="scr_in", tag="scr_in")
            scr = const.tile([1, 16], F32, name="scr", tag="scr")
            bqk_sb = const.tile([128, 4, 1], F32, name="bqk_sb", tag="bqk_sb")
            bv1 = const.tile([1, F], F32, name="bv1", tag="bv1")
            bvb = const.tile([128, F], F32, name="bvb", tag="bvb")

            # x^T, c-major: [p, c, tok] (qk rhs / v lhsT), bf16 direct DMA
            xT4 = persist.tile([128, CT, N], BF16, name="xT4", tag="xT4")
            # qkT[0..1] = Q^T head-pairs, qkT[2..3] = K^T head-pairs
            qkT = [persist.tile([128, N], BF16, name=f"qkT{f}", tag=f"qkT{f}")
                   for f in range(4)]
            vaug = [persist.tile([128, 65 * HPC], BF16, name=f"vaug{t}",
                                 tag=f"vaug{t}")
                    for t in range(NT)]
            # O^T stacked per head pair (rows 0-63 head 2p, 64-127 head 2p+1)
            oT2 = [persist.tile([128, N], BF16, name=f"oT2{p}", tag=f"oT2{p}")
                   for p in range(2)]
            wqk = persist.tile([128, CT, 2 * F], BF16, name="wqk", tag="wqk")
            wv = persist.tile([128, CT, F], BF16, name="wv", tag="wv")
            wp2 = persist.tile([128, 2, C], BF16, name="wp2", tag="wp2")

            ptp = ctx.enter_context(tc.tile_pool(name="ptp", bufs=16))
            ysb = ctx.enter_context(tc.tile_pool(name="ysb", bufs=2))
            snr = ctx.enter_context(tc.tile_pool(name="snr", bufs=6))
            snb = ctx.enter_context(tc.tile_pool(name="snb", bufs=2))
            sev = ctx.enter_context(tc.tile_pool(name="sev", bufs=8))

            bpp = ctx.enter_context(
                tc.tile_pool(name="bpp", bufs=2, space=bass.MemorySpace.PSUM))
            cpp = ctx.enter_context(
                tc.tile_pool(name="cpp", bufs=2, space=bass.MemorySpace.PSUM))
            aux = ctx.enter_context(
                tc.tile_pool(name="aux", bufs=2, space=bass.MemorySpace.PSUM))

            # ---------------- boot: warmup + exp table preload ----------
            nc.vector.memset(warm[:], 0.0)
            nc.vector.memset(ones1[:], 1.0)
            nc.vector.memset(scr_in[:], 0.0)
            nc.scalar.activation(scr[:], scr_in[:], AF.Exp)
            for _ in range(NWARM):
                wa = aux.tile([128, 512], F32, name="wa", tag="aux")
                nc.tensor.matmul(wa[:], warm[:, 0:128], warm[:],
                                 start=True, stop=True)

            # ---------------- DMA issue (deadline order per ring) -------
            xt = xt_d.ap().rearrange("(c p) n -> p c n", p=128)
            wqk_v = wqk_d.ap().rearrange("(c p) f -> p c f", p=128)
            wv_v = wv_d.ap().rearrange("(c p) f -> p c f", p=128)
            wp_v = wp_d.ap().rearrange("(t p) f -> p t f", p=128)
            bqk_v = bqk_d.ap().rearrange("(g p) o -> p g o", p=128)

            # NOTE: all transfers use 2D [128, n] dest slices — big 3D
            # strided DMAs were observed to break the DMA->reader RAW
            # dependency on hw (readers ran on stale SBUF)
            # scalar ring (HWDGE): tiny biases + prefix-critical wqk + ch1
            for g in range(4):
                nc.scalar.dma_start(bqk_sb[:, g], bqk_v[:, g])
            nc.scalar.dma_start(bv1[:], bv_d.ap()[:])
            for c in range(6):
                nc.scalar.dma_start(wqk[:, c], wqk_v[:, c])
            for c in range(4):
                nc.scalar.dma_start(xT4[:, c, 512:1024], xt[:, c, 512:1024])
            # sync ring (HWDGE): x first token chunk (c 0-3) + later chunks
            for c in range(4):
                nc.sync.dma_start(xT4[:, c, 0:512], xt[:, c, 0:512])
            for c in range(4):
                nc.sync.dma_start(xT4[:, c, 1024:1536], xt[:, c, 1024:1536])
            for c in range(4):
                nc.sync.dma_start(xT4[:, c, 1536:2048], xt[:, c, 1536:2048])
            # gpsimd ring (SWDGE): x (c 4-7), rest of wqk, wv, wp
            for c in range(4, 8):
                nc.gpsimd.dma_start(xT4[:, c, 0:512], xt[:, c, 0:512])
            for c in (6, 7):
                nc.gpsimd.dma_start(wqk[:, c], wqk_v[:, c])
            for c in range(4, 8):
                nc.gpsimd.dma_start(xT4[:, c, 512:1024], xt[:, c, 512:1024])
            for c in range(CT):
                nc.gpsimd.dma_start(wv[:, c], wv_v[:, c])
            for c in range(4, 8):
                nc.gpsimd.dma_start(xT4[:, c, 1024:1536], xt[:, c, 1024:1536])
            for c in range(4, 8):
                nc.gpsimd.dma_start(xT4[:, c, 1536:2048], xt[:, c, 1536:2048])
            for p in range(2):
                nc.gpsimd.dma_start(wp2[:, p], wp_v[:, p])
            nc.gpsimd.partition_broadcast(bvb[:], bv1[:])

            # ---------------- helper emitters ----------------
            def qk_unit(f, ch):
                # qkT[f][:, ch*512:(ch+1)*512] = (wqk_f^T @ x^T) + bias
                qp = aux.tile([128, 512], F32, name="qp", tag="aux")
                for c in (0, 4, 1, 5, 2, 6, 3, 7):
                    nc.tensor.matmul(
                        qp[:],
                        wqk[:, c, f * 128:(f + 1) * 128],
                        xT4[:, c, ch * 512:(ch + 1) * 512],
                        start=(c == 0), stop=(c == 7))
                nc.vector.tensor_scalar_add(
                    qkT[f][:, ch * 512:(ch + 1) * 512], qp[:],
                    bqk_sb[:, f, 0:1])

            vaug_w = {}

            def v_unit(t):
                # vaug[t] cols [65h,65h+64] = V head h | ones column.
                # The strided add write is invisible to the dep tracker
                # (like all non-plain writes): PV readers take manual
                # edges via vaug_w.
                vp = aux.tile([128, F], F32, name="vp", tag="aux")
                for c in range(CT):
                    nc.tensor.matmul(
                        vp[:], xT4[:, c, t * 128:(t + 1) * 128], wv[:, c],
                        start=(c == 0), stop=(c == CT - 1))
                ws = [nc.vector.memset(vaug[t][:, 65 * h + 64:65 * h + 65], 1.0)
                      for h in range(HPC)]
                vv = vaug[t].rearrange("p (h d) -> p h d", h=HPC)
                ws.append(nc.vector.tensor_add(
                    vv[:, :, 0:D],
                    vp.rearrange("p (h d) -> p h d", h=HPC),
                    bvb.rearrange("p (h d) -> p h d", h=HPC)))
                vaug_w[t] = ws

            def yp_unit(t):
                # one DVE writer -> one sync-ring DMA per ys tile (the
                # baseline-proven output pattern). The oT2 columns this
                # reads were written by partition-sliced muls the
                # tracker can't see, so add manual RAW edges.
                for ch in range(2):
                    yp = aux.tile([128, 512], F32, name="yp", tag="aux")
                    for p in range(2):
                        m = nc.tensor.matmul(
                            yp[:],
                            oT2[p][:, t * 128:(t + 1) * 128],
                            wp2[:, p, ch * 512:(ch + 1) * 512],
                            start=(p == 0), stop=(p == 1))
                        for mu in o_muls[t // 4]:
                            add_dep_helper(m.ins, mu.ins, reason="oT2->yp")
                    ys = ysb.tile([128, 512], BF16, name="ys", tag="ys")
                    nc.vector.tensor_copy(ys[:], yp[:])
                    nc.sync.dma_start(
                        y_d.ap()[t * 128:(t + 1) * 128,
                                 ch * 512:(ch + 1) * 512], ys[:])

            # -------- filler queue (deadline ordered) -----
            # slot = global key-tile index 0..63; fillers popped one per
            # slot plus forced pops when a deadline is due
            fillers = []

            def defer(dl, fn, *a, nb=0):
                fillers.append((dl, nb, lambda: fn(*a)))

            # deadlines in global unit slots (8 pair-quarters x 16):
            # K-pair0 chunks feed pq0, K-pair1 feed pq1; Q chunks feed
            # their pair-quarter; v tiles feed PV at lag; yp after the
            # producing quarter's second normalize (not_before)
            defer(3, qk_unit, 2, 1)
            defer(7, qk_unit, 2, 2)
            defer(11, qk_unit, 2, 3)
            defer(12, qk_unit, 3, 0)
            defer(14, qk_unit, 1, 0)
            defer(19, qk_unit, 3, 1)
            defer(23, qk_unit, 3, 2)
            defer(27, qk_unit, 3, 3)
            defer(30, qk_unit, 0, 1)
            defer(45, qk_unit, 1, 1)
            defer(62, qk_unit, 0, 2)
            defer(77, qk_unit, 1, 2)
            defer(94, qk_unit, 0, 3)
            defer(109, qk_unit, 1, 3)
            for t in range(NT):
                defer(t + PVLAG - 1, v_unit, t)
            for t in range(12):
                defer(32 * (t // 4 + 1) + 8 + (t % 4), yp_unit, t,
                      nb=32 * (t // 4 + 1) + 6)
            fillers.sort(key=lambda x: x[0])

            def emit_fillers(slot):
                popped = False
                while fillers:
                    ready = [f for f in fillers if f[1] <= slot]
                    if not ready:
                        break
                    due = [f for f in ready if f[0] <= slot]
                    if due:
                        f = due[0]
                    elif not popped:
                        f = ready[0]
                    else:
                        break
                    fillers.remove(f)
                    f[2]()
                    popped = True

            # ---------------- prefix: K/Q chunk 0, pair 0 ----------
            qk_unit(2, 0)
            qk_unit(0, 0)

            # ---------------- fused attention ----------------
            # Exactly ONE psum accumulation group per bank (multiple
            # start=True groups in one bank are illegal: the clear is
            # bank-granular). Denominators ride the ones column as row
            # 64 of each [65, 512] accumulator.
            # The tracker is blind to PARTITION-SLICED WRITES (the oT2
            # muls): their readers (yp matmuls) and the cp-bank reuse
            # WAR get explicit add_dep_helper edges.
            cp_mms = [[], []]
            o_muls = {}
            war_deps = []

            def pv_unit(ent, cph, cphp, pair):
                pt, mt = ent
                st, sp = (mt == 0), (mt == NT - 1)
                he = 2 * pair
                m0 = nc.tensor.matmul(
                    cph[:], vaug[mt][:, 65 * he:65 * he + 65], pt[:, 0],
                    start=st, stop=sp)
                m1 = nc.tensor.matmul(
                    cphp[:], vaug[mt][:, 65 * he + 65:65 * he + 130], pt[:, 1],
                    start=st, stop=sp)
                for m in (m0, m1):
                    for w in vaug_w[mt]:
                        add_dep_helper(m.ins, w.ins, reason="vaug->pv")
                    if mt == 0:
                        for w in war_deps:
                            add_dep_helper(m.ins, w.ins, reason="evac->next")
                cp_mms[0].append(m0)
                cp_mms[1].append(m1)

            def normalize(quarter, pair, qs, cph, cphp):
                # evacuate both accumulators to sbuf with full-tile
                # copies (tracked), then normalize from sbuf
                sc = [sev.tile([65, 512], F32, name=f"sc{i}", tag="sev")
                      for i in range(2)]
                i_sc = [nc.vector.tensor_copy(sc[0][:], cph[:]),
                        nc.vector.tensor_copy(sc[1][:], cphp[:])]
                for i in range(2):
                    for m in cp_mms[i]:
                        add_dep_helper(i_sc[i].ins, m.ins, reason="cp->evac")
                muls = o_muls.setdefault(quarter, [])
                for i in range(2):
                    h = 2 * pair + i
                    rb = 64 * (h % 2)
                    d0 = snr.tile([1, 512], F32, name="d0", tag="sr")
                    nc.vector.tensor_copy(d0[:], sc[i][64:65, :])
                    sr = snr.tile([1, 512], F32, name="sr", tag="sr")
                    nc.vector.reciprocal_approx_fast(sr[:], d0[:])
                    sb = snb.tile([128, 512], F32, name="sb", tag="sb")
                    nc.gpsimd.partition_broadcast(sb[:], sr[:])
                    muls.append(nc.vector.tensor_mul(
                        oT2[h // 2][rb:rb + 64, qs:qs + 512],
                        sc[i][0:64, :], sb[0:64, :]))
                war_deps[:] = [i_sc[0], i_sc[1]]
                cp_mms[0] = []
                cp_mms[1] = []

            pts = []
            cp_cur = {}

            def pop_pv():
                # PV lag carries ACROSS pair-quarter boundaries: cp
                # tiles allocate lazily at mt==0 and normalize fires
                # right when mt==15 pops, so no drain wall ever sits in
                # front of the next quarter's logits on the PE queue
                pt, mt, pq2 = pts.pop(0)
                q2, p2 = divmod(pq2, 2)
                if mt == 0:
                    cp_cur[pq2] = (
                        cpp.tile([65, 512], F32, name="cph", tag="cp"),
                        cpp.tile([65, 512], F32, name="cphp", tag="cp"))
                cph, cphp = cp_cur[pq2]
                pv_unit((pt, mt), cph, cphp, p2)
                if mt == NT - 1:
                    normalize(q2, p2, q2 * 512, cph, cphp)

            for pq in range(8):
                quarter, pair = divmod(pq, 2)
                qs = quarter * 512
                for mt in range(NT):
                    slot = pq * NT + mt
                    bp = bpp.tile([128, 2, 512], F32, name="bp", tag="bp")
                    nc.tensor.matmul(
                        bp[:, 0], qkT[2 + pair][0:64, mt * 128:(mt + 1) * 128],
                        qkT[pair][0:64, qs:qs + 512], start=True, stop=True)
                    nc.tensor.matmul(
                        bp[:, 1], qkT[2 + pair][64:128, mt * 128:(mt + 1) * 128],
                        qkT[pair][64:128, qs:qs + 512], start=True, stop=True)
                    pt = ptp.tile([128, 2, 512], BF16, name="pt", tag="pt")
                    nc.scalar.activation(pt[:], bp[:], AF.Exp, scale=scale)
                    pts.append((pt, mt, pq))
                    if len(pts) > PVLAG:
                        pop_pv()
                    emit_fillers(slot)
            while pts:
                pop_pv()

            # tail
            while fillers:
                fillers.pop(0)[2]()
            for t in range(12, 16):
                yp_unit(t)

    nc.compile()
    return nc


def _get_nc():
    if "nc" not in _CACHE:
        _CACHE["nc"] = _build()
    return _CACHE["nc"]


def _in_maps(q, W_qkv, b_qkv, W_proj):
    bf16 = ml_dtypes.bfloat16
    # shared across cores: x^T per batch, per-group weight slices
    xts = [np.ascontiguousarray(np.asarray(q[b]).T).astype(bf16)
           for b in range(B)]
    wqks, wvs, wps, bqks, bvs = [], [], [], [], []
    for g in range(HPC):
        cols = slice(g * F, (g + 1) * F)
        wqks.append(np.ascontiguousarray(
            np.concatenate([W_qkv[:, cols], W_qkv[:, C:2 * C][:, cols]],
                           axis=1)).astype(bf16))
        wvs.append(np.ascontiguousarray(W_qkv[:, 2 * C:][:, cols]).astype(bf16))
        wps.append(np.ascontiguousarray(W_proj[cols, :]).astype(bf16))
        bqks.append(np.ascontiguousarray(
            np.concatenate([b_qkv[cols], b_qkv[C:2 * C][cols]])
            .reshape(2 * F, 1).astype(np.float32)))
        bvs.append(np.ascontiguousarray(
            b_qkv[2 * C:][cols].reshape(1, F).astype(np.float32)))
    maps = []
    for core in range(NCORES):
        b, g = divmod(core, HPC)
        maps.append({
            "xt": xts[b],
            "wqk": wqks[g],
            "wv": wvs[g],
            "wp": wps[g],
            "bqk": bqks[g],
            "bv": bvs[g],
        })
    return maps


def kernel(q, W_qkv, b_qkv, W_proj, b_proj):
    from concourse.bass_utils import run_bass_kernel_spmd

    q = np.asarray(q, dtype=np.float32)
    W_qkv = np.asarray(W_qkv, dtype=np.float32)
    b_qkv = np.asarray(b_qkv, dtype=np.float32)
    W_proj = np.asarray(W_proj, dtype=np.float32)
    b_proj = np.asarray(b_proj, dtype=np.float32)

    nc = _get_nc()
    res = run_bass_kernel_spmd(nc, _in_maps(q, W_qkv, b_qkv, W_proj),
                               core_ids=list(range(NCORES)))

    out = np.zeros((B, N, C), dtype=np.float32)
    for core in range(NCORES):
        out[core // HPC] += np.asarray(res.results[core]["y"], dtype=np.float32)
    out += b_proj
    return out


# revision 33
# speedup vs baseline: 1.0757x; 1.0282x over previous
"""Trainium2 Bass kernel for nn_Attention_56470230008033.

Multi-head self-attention (B=2, N=2048, C=1024, H=16 heads, D=64),
k = v = q, full qkv projection + output projection.

Sharding over 8 NeuronCores: data parallel on batch (2) x tensor
parallel on heads (4 head-groups of 4 heads).

v5: ScalarE-exp-bound pipeline (~147us of ACTIVATE is the floor).
  - host pre-transposes x and pre-casts x/weights to bf16: no PE
    transposes, no DVE weight/x casts, input DMA halved (6MB)
  - pair-sequential units (8 pair-quarters x 16 key tiles): logits
    as one row-paired wall (2 heads via tile_position), ONE exp per
    unit (FD=1024, back-to-back 1.15us stream), PV as 2x [65,512]
    ones-column matmuls (denominator rides the 65th row; exactly one
    accumulation group per psum bank - multiple groups per bank are
    illegal, the hw start-flag clear is bank-granular)
  - PSUM: bp 2x[128,2,512] (exp double buffer) + cp 2x[65,512] +
    aux 2 = 8 banks
  - the tile framework does not track partition-sliced WRITES (the
    oT2 normalize muls): their readers get explicit add_dep_helper
    edges, as does the cp-bank reuse WAR across pair-quarters
  - input DMA split across sync/scalar/gpsimd rings, 2D descriptors,
    deadline-ordered; ScalarE's ring is idle before the exp stream
  - ~5us of dummy matmuls at boot warm the PE HAM clock gate
  - qkv/proj matmuls, v-units and output tiles run as deadline-paced
    fillers in the PE slack under the exp stream
"""

import sys

for _p in ("/opt/trn_rl_repo", "/opt/pypackages"):
    if _p not in sys.path:
        sys.path.append(_p)

import numpy as np
import ml_dtypes

B, N, C, H = 2, 2048, 1024, 16
D = C // H            # 64 head dim
NCORES = 8
HPC = 4               # heads per core
F = HPC * D           # 256 features per core
NT = N // 128         # 16 token tiles
CT = C // 128         # 8 contraction tiles

PVLAG = 10            # PV lag in key-tile units
NWARM = 48            # dummy matmuls to warm the PE clock gate

_CACHE = {}


def _build():
    from concourse import bacc, bass, mybir, tile
    from concourse.tile import add_dep_helper

    F32 = mybir.dt.float32
    BF16 = mybir.dt.bfloat16
    AF = mybir.ActivationFunctionType

    nc = bacc.Bacc(
        "TRN2",
        target_bir_lowering=False,
        debug=False,
        enable_asserts=False,
        num_devices=NCORES,
    )
    xt_d = nc.dram_tensor("xt", [C, N], BF16, kind="ExternalInput")
    wqk_d = nc.dram_tensor("wqk", [C, 2 * F], BF16, kind="ExternalInput")
    wv_d = nc.dram_tensor("wv", [C, F], BF16, kind="ExternalInput")
    wp_d = nc.dram_tensor("wp", [F, C], BF16, kind="ExternalInput")
    bqk_d = nc.dram_tensor("bqk", [2 * F, 1], F32, kind="ExternalInput")
    bv_d = nc.dram_tensor("bv", [1, F], F32, kind="ExternalInput")
    y_d = nc.dram_tensor("y", [N, C], BF16, kind="ExternalOutput")

    scale = float(D) ** -0.5

    with tile.TileContext(nc) as tc:
        from contextlib import ExitStack

        with ExitStack() as ctx:
            const = ctx.enter_context(tc.tile_pool(name="const", bufs=1))
            persist = ctx.enter_context(tc.tile_pool(name="persist", bufs=1))

            warm = const.tile([128, 512], BF16, name="warm", tag="warm")
            ones1 = const.tile([128, 1], BF16, name="ones1", tag="ones1")
            scr_in = const.tile([1, 16], F32, name="scr_in", tag="scr_in")
            scr = const.tile([1, 16], F32, name="scr", tag="scr")
            bqk_sb = const.tile([128, 4, 1], F32, name="bqk_sb", tag="bqk_sb")
            bv1 = const.tile([1, F], F32, name="bv1", tag="bv1")
            bvb = const.tile([128, F], F32, name="bvb", tag="bvb")

            # x^T, c-major: [p, c, tok] (qk rhs / v lhsT), bf16 direct DMA
            xT4 = persist.tile([128, CT, N], BF16, name="xT4", tag="xT4")
            # qkT[0..1] = Q^T head-pairs, qkT[2..3] = K^T head-pairs
            qkT = [persist.tile([128, N], BF16, name=f"qkT{f}", tag=f"qkT{f}")
                   for f in range(4)]
            vaug = [persist.tile([128, 65 * HPC], BF16, name=f"vaug{t}",
                                 tag=f"vaug{t}")
                    for t in range(NT)]
            # O^T stacked per head pair (rows 0-63 head 2p, 64-127 head 2p+1)
            oT2 = [persist.tile([128, N], BF16, name=f"oT2{p}", tag=f"oT2{p}")
                   for p in range(2)]
            wqk = persist.tile([128, CT, 2 * F], BF16, name="wqk", tag="wqk")
            wv = persist.tile([128, CT, F], BF16, name="wv", tag="wv")
            wp2 = persist.tile([128, 2, C], BF16, name="wp2", tag="wp2")

            ptp = ctx.enter_context(tc.tile_pool(name="ptp", bufs=16))
            ysb = ctx.enter_context(tc.tile_pool(name="ysb", bufs=2))
            snr = ctx.enter_context(tc.tile_pool(name="snr", bufs=6))
            snb = ctx.enter_context(tc.tile_pool(name="snb", bufs=2))
            sev = ctx.enter_context(tc.tile_pool(name="sev", bufs=8))

            bpp = ctx.enter_context(
                tc.tile_pool(name="bpp", bufs=2, space=bass.MemorySpace.PSUM))
            cpp = ctx.enter_context(
                tc.tile_pool(name="cpp", bufs=2, space=bass.MemorySpace.PSUM))
            aux = ctx.enter_context(
                tc.tile_pool(name="aux", bufs=2, space=bass.MemorySpace.PSUM))

            # ---------------- boot: warmup + exp table preload ----------
            nc.vector.memset(warm[:], 0.0)
            nc.vector.memset(ones1[:], 1.0)
            nc.vector.memset(scr_in[:], 0.0)
            nc.scalar.activation(scr[:], scr_in[:], AF.Exp)
            for _ in range(NWARM):
                wa = aux.tile([128, 512], F32, name="wa", tag="aux")
                nc.tensor.matmul(wa[:], warm[:, 0:128], warm[:],
                                 start=True, stop=True)

            # ---------------- DMA issue (deadline order per ring) -------
            xt = xt_d.ap().rearrange("(c p) n -> p c n", p=128)
            wqk_v = wqk_d.ap().rearrange("(c p) f -> p c f", p=128)
            wv_v = wv_d.ap().rearrange("(c p) f -> p c f", p=128)
            wp_v = wp_d.ap().rearrange("(t p) f -> p t f", p=128)
            bqk_v = bqk_d.ap().rearrange("(g p) o -> p g o", p=128)

            # NOTE: all transfers use 2D [128, n] dest slices — big 3D
            # strided DMAs were observed to break the DMA->reader RAW
            # dependency on hw (readers ran on stale SBUF)
            # scalar ring (HWDGE): tiny biases + prefix-critical wqk + ch1
            for g in range(4):
                nc.scalar.dma_start(bqk_sb[:, g], bqk_v[:, g])
            nc.scalar.dma_start(bv1[:], bv_d.ap()[:])
            for c in range(6):
                nc.scalar.dma_start(wqk[:, c], wqk_v[:, c])
            for c in range(4):
                nc.scalar.dma_start(xT4[:, c, 512:1024], xt[:, c, 512:1024])
            # sync ring (HWDGE): x first token chunk (c 0-3) + later chunks
            for c in range(4):
                nc.sync.dma_start(xT4[:, c, 0:512], xt[:, c, 0:512])
            for c in range(4):
                nc.sync.dma_start(xT4[:, c, 1024:1536], xt[:, c, 1024:1536])
            for c in range(4):
                nc.sync.dma_start(xT4[:, c, 1536:2048], xt[:, c, 1536:2048])
            # gpsimd ring (SWDGE): x (c 4-7), rest of wqk, wv, wp
            for c in range(4, 8):
                nc.gpsimd.dma_start(xT4[:, c, 0:512], xt[:, c, 0:512])
            for c in (6, 7):
                nc.gpsimd.dma_start(wqk[:, c], wqk_v[:, c])
            for c in range(4, 8):
                nc.gpsimd.dma_start(xT4[:, c, 512:1024], xt[:, c, 512:1024])
            for c in range(CT):
                nc.gpsimd.dma_start(wv[:, c], wv_v[:, c])
            for c in range(4, 8):
                nc.gpsimd.dma_start(xT4[:, c, 1024:1536], xt[:, c, 1024:1536])
            for c in range(4, 8):
                nc.gpsimd.dma_start(xT4[:, c, 1536:2048], xt[:, c, 1536:2048])
            for p in range(2):
                nc.gpsimd.dma_start(wp2[:, p], wp_v[:, p])
            nc.gpsimd.partition_broadcast(bvb[:], bv1[:])

            # ---------------- helper emitters ----------------
            def qk_unit(f, ch):
                # qkT[f][:, ch*512:(ch+1)*512] = (wqk_f^T @ x^T) + bias
                qp = aux.tile([128, 512], F32, name="qp", tag="aux")
                for c in (0, 4, 1, 5, 2, 6, 3, 7):
                    nc.tensor.matmul(
                        qp[:],
                        wqk[:, c, f * 128:(f + 1) * 128],
                        xT4[:, c, ch * 512:(ch + 1) * 512],
                        start=(c == 0), stop=(c == 7))
                nc.vector.tensor_scalar_add(
                    qkT[f][:, ch * 512:(ch + 1) * 512], qp[:],
                    bqk_sb[:, f, 0:1])

            vaug_w = {}

            def v_unit(t):
                # vaug[t] cols [65h,65h+64] = V head h | ones column.
                # The strided add write is invisible to the dep tracker
                # (like all non-plain writes): PV readers take manual
                # edges via vaug_w.
                vp = aux.tile([128, F], F32, name="vp", tag="aux")
                for c in range(CT):
                    nc.tensor.matmul(
                        vp[:], xT4[:, c, t * 128:(t + 1) * 128], wv[:, c],
                        start=(c == 0), stop=(c == CT - 1))
                ws = [nc.vector.memset(vaug[t][:, 65 * h + 64:65 * h + 65], 1.0)
                      for h in range(HPC)]
                vv = vaug[t].rearrange("p (h d) -> p h d", h=HPC)
                ws.append(nc.vector.tensor_add(
                    vv[:, :, 0:D],
                    vp.rearrange("p (h d) -> p h d", h=HPC),
                    bvb.rearrange("p (h d) -> p h d", h=HPC)))
                vaug_w[t] = ws

            def yp_unit(t):
                # one DVE writer -> one sync-ring DMA per ys tile (the
                # baseline-proven output pattern). The oT2 columns this
                # reads were written by partition-sliced muls the
                # tracker can't see, so add manual RAW edges.
                for ch in range(2):
                    yp = aux.tile([128, 512], F32, name="yp", tag="aux")
                    for p in range(2):
                        m = nc.tensor.matmul(
                            yp[:],
                            oT2[p][:, t * 128:(t + 1) * 128],
                            wp2[:, p, ch * 512:(ch + 1) * 512],
                            start=(p == 0), stop=(p == 1))
                        for mu in o_muls[t // 4]:
                            add_dep_helper(m.ins, mu.ins, reason="oT2->yp")
                    ys = ysb.tile([128, 512], BF16, name="ys", tag="ys")
                    nc.vector.tensor_copy(ys[:], yp[:])
                    nc.sync.dma_start(
                        y_d.ap()[t * 128:(t + 1) * 128,
                                 ch * 512:(ch + 1) * 512], ys[:])

            # -------- filler queue (deadline ordered) -----
            # slot = global key-tile index 0..63; fillers popped one per
            # slot plus forced pops when a deadline is due
            fillers = []

            def defer(dl, fn, *a, nb=0):
                fillers.append((dl, nb, lambda: fn(*a)))

            # deadlines in global unit slots (8 pair-quarters x 16):
            # K-pair0 chunks feed pq0, K-pair1 feed pq1; Q chunks feed
            # their pair-quarter; v tiles feed PV at lag; yp after the
            # producing quarter's second normalize (not_before)
            defer(3, qk_unit, 2, 1)
            defer(7, qk_unit, 2, 2)
            defer(11, qk_unit, 2, 3)
            defer(12, qk_unit, 3, 0)
            defer(14, qk_unit, 1, 0)
            defer(19, qk_unit, 3, 1)
            defer(23, qk_unit, 3, 2)
            defer(27, qk_unit, 3, 3)
            defer(30, qk_unit, 0, 1)
            defer(45, qk_unit, 1, 1)
            defer(62, qk_unit, 0, 2)
            defer(77, qk_unit, 1, 2)
            defer(94, qk_unit, 0, 3)
            defer(109, qk_unit, 1, 3)
            for t in range(NT):
                defer(t + PVLAG - 1, v_unit, t)
            for t in range(12):
                # normalize(q, pair1) pops PVLAG slots after its last
                # unit (global slot 32(q+1)-1+PVLAG); yp must not be
                # emitted before it, so its o_muls edges cover BOTH
                # pairs' oT2 writes
                defer(32 * (t // 4 + 1) + PVLAG + 7 + (t % 4), yp_unit, t,
                      nb=32 * (t // 4 + 1) + PVLAG + 5)
            fillers.sort(key=lambda x: x[0])

            def emit_fillers(slot):
                popped = False
                while fillers:
                    ready = [f for f in fillers if f[1] <= slot]
                    if not ready:
                        break
                    due = [f for f in ready if f[0] <= slot]
                    if due:
                        f = due[0]
                    elif not popped:
                        f = ready[0]
                    else:
                        break
                    fillers.remove(f)
                    f[2]()
                    popped = True

            # ---------------- prefix: K/Q chunk 0, pair 0 ----------
            qk_unit(2, 0)
            qk_unit(0, 0)

            # ---------------- fused attention ----------------
            # Exactly ONE psum accumulation group per bank (multiple
            # start=True groups in one bank are illegal: the clear is
            # bank-granular). Denominators ride the ones column as row
            # 64 of each [65, 512] accumulator.
            # The tracker is blind to PARTITION-SLICED WRITES (the oT2
            # muls): their readers (yp matmuls) and the cp-bank reuse
            # WAR get explicit add_dep_helper edges.
            cp_mms = [[], []]
            o_muls = {}
            war_deps = []

            def pv_unit(ent, cph, cphp, pair):
                pt, mt = ent
                st, sp = (mt == 0), (mt == NT - 1)
                he = 2 * pair
                m0 = nc.tensor.matmul(
                    cph[:], vaug[mt][:, 65 * he:65 * he + 65], pt[:, 0],
                    start=st, stop=sp)
                m1 = nc.tensor.matmul(
                    cphp[:], vaug[mt][:, 65 * he + 65:65 * he + 130], pt[:, 1],
                    start=st, stop=sp)
                for m in (m0, m1):
                    for w in vaug_w[mt]:
                        add_dep_helper(m.ins, w.ins, reason="vaug->pv")
                    if mt == 0:
                        for w in war_deps:
                            add_dep_helper(m.ins, w.ins, reason="evac->next")
                cp_mms[0].append(m0)
                cp_mms[1].append(m1)

            def normalize(quarter, pair, qs, cph, cphp):
                # evacuate both accumulators to sbuf with full-tile
                # copies (tracked), then normalize from sbuf
                sc = [sev.tile([65, 512], F32, name=f"sc{i}", tag="sev")
                      for i in range(2)]
                i_sc = [nc.vector.tensor_copy(sc[0][:], cph[:]),
                        nc.vector.tensor_copy(sc[1][:], cphp[:])]
                for i in range(2):
                    for m in cp_mms[i]:
                        add_dep_helper(i_sc[i].ins, m.ins, reason="cp->evac")
                muls = o_muls.setdefault(quarter, [])
                for i in range(2):
                    h = 2 * pair + i
                    rb = 64 * (h % 2)
                    d0 = snr.tile([1, 512], F32, name="d0", tag="sr")
                    nc.vector.tensor_copy(d0[:], sc[i][64:65, :])
                    sr = snr.tile([1, 512], F32, name="sr", tag="sr")
                    nc.vector.reciprocal_approx_fast(sr[:], d0[:])
                    sb = snb.tile([128, 512], F32, name="sb", tag="sb")
                    nc.gpsimd.partition_broadcast(sb[:], sr[:])
                    muls.append(nc.vector.tensor_mul(
                        oT2[h // 2][rb:rb + 64, qs:qs + 512],
                        sc[i][0:64, :], sb[0:64, :]))
                war_deps[:] = [i_sc[0], i_sc[1]]
                cp_mms[0] = []
                cp_mms[1] = []

            pts = []
            cp_cur = {}

            def pop_pv():
                # PV lag carries ACROSS pair-quarter boundaries: cp
                # tiles allocate lazily at mt==0 and normalize fires
                # right when mt==15 pops, so no drain wall ever sits in
                # front of the next quarter's logits on the PE queue
                pt, mt, pq2 = pts.pop(0)
                q2, p2 = divmod(pq2, 2)
                if mt == 0:
                    cp_cur[pq2] = (
                        cpp.tile([65, 512], F32, name="cph", tag="cp"),
                        cpp.tile([65, 512], F32, name="cphp", tag="cp"))
                cph, cphp = cp_cur[pq2]
                pv_unit((pt, mt), cph, cphp, p2)
                if mt == NT - 1:
                    normalize(q2, p2, q2 * 512, cph, cphp)

            for pq in range(8):
                quarter, pair = divmod(pq, 2)
                qs = quarter * 512
                for mt in range(NT):
                    slot = pq * NT + mt
                    bp = bpp.tile([128, 2, 512], F32, name="bp", tag="bp")
                    nc.tensor.matmul(
                        bp[:, 0], qkT[2 + pair][0:64, mt * 128:(mt + 1) * 128],
                        qkT[pair][0:64, qs:qs + 512], start=True, stop=True)
                    nc.tensor.matmul(
                        bp[:, 1], qkT[2 + pair][64:128, mt * 128:(mt + 1) * 128],
                        qkT[pair][64:128, qs:qs + 512], start=True, stop=True)
                    pt = ptp.tile([128, 2, 512], BF16, name="pt", tag="pt")
                    nc.scalar.activation(pt[:], bp[:], AF.Exp, scale=scale)
                    pts.append((pt, mt, pq))
                    if len(pts) > PVLAG:
                        pop_pv()
                    emit_fillers(slot)
            while pts:
                pop_pv()

            # tail
            while fillers:
                fillers.pop(0)[2]()
            for t in range(12, 16):
                yp_unit(t)

    nc.compile()
    return nc


def _get_nc():
    if "nc" not in _CACHE:
        _CACHE["nc"] = _build()
    return _CACHE["nc"]


def _in_maps(q, W_qkv, b_qkv, W_proj):
    bf16 = ml_dtypes.bfloat16
    # shared across cores: x^T per batch, per-group weight slices
    xts = [np.ascontiguousarray(np.asarray(q[b]).T).astype(bf16)
           for b in range(B)]
    wqks, wvs, wps, bqks, bvs = [], [], [], [], []
    for g in range(HPC):
        cols = slice(g * F, (g + 1) * F)
        wqks.append(np.ascontiguousarray(
            np.concatenate([W_qkv[:, cols], W_qkv[:, C:2 * C][:, cols]],
                           axis=1)).astype(bf16))
        wvs.append(np.ascontiguousarray(W_qkv[:, 2 * C:][:, cols]).astype(bf16))
        wps.append(np.ascontiguousarray(W_proj[cols, :]).astype(bf16))
        bqks.append(np.ascontiguousarray(
            np.concatenate([b_qkv[cols], b_qkv[C:2 * C][cols]])
            .reshape(2 * F, 1).astype(np.float32)))
        bvs.append(np.ascontiguousarray(
            b_qkv[2 * C:][cols].reshape(1, F).astype(np.float32)))
    maps = []
    for core in range(NCORES):
        b, g = divmod(core, HPC)
        maps.append({
            "xt": xts[b],
            "wqk": wqks[g],
            "wv": wvs[g],
            "wp": wps[g],
            "bqk": bqks[g],
            "bv": bvs[g],
        })
    return maps


def kernel(q, W_qkv, b_qkv, W_proj, b_proj):
    from concourse.bass_utils import run_bass_kernel_spmd

    q = np.asarray(q, dtype=np.float32)
    W_qkv = np.asarray(W_qkv, dtype=np.float32)
    b_qkv = np.asarray(b_qkv, dtype=np.float32)
    W_proj = np.asarray(W_proj, dtype=np.float32)
    b_proj = np.asarray(b_proj, dtype=np.float32)

    nc = _get_nc()
    res = run_bass_kernel_spmd(nc, _in_maps(q, W_qkv, b_qkv, W_proj),
                               core_ids=list(range(NCORES)))

    out = np.zeros((B, N, C), dtype=np.float32)
    for core in range(NCORES):
        out[core // HPC] += np.asarray(res.results[core]["y"], dtype=np.float32)
    out += b_proj
    return out
